# revision 4
# baseline (speedup 1.0000x reference)
"""DGCNN forward kernel v3 for Trainium2 (8 NeuronCores, data-parallel over batch).

Each core processes one point cloud (N=2048 points) end to end:
  4x EdgeConv (KNN k=20 + 1x1 conv + BN + LeakyReLU(0.2) + max over k)
  -> concat -> 1x1 conv to 1024 + BN + LeakyReLU -> global max+mean pool
  -> MLP 2048-512-256-128-2 with LeakyReLU(0.01).

Key algebraic rewrite: for monotone BN (scale>0) and LeakyReLU,
  max_k f(W @ [nbr - ctr, ctr]) = f(max_k(U[idx_k]) + V),
with U = Wl @ x, V = (Wr - Wl) @ x.

v3 vs v1: the edge-conv tile loop is software-pipelined so that the three
slow per-tile stages run on different engines concurrently:
  - PE computes tile t+1's S-matrix while
  - DVE runs tile t's top-k (max8/find_index8/match_replace8) while
  - GpSimd issues tile t-1's dma_gather and
  - DVE k-reduces tile t-2's gathered rows (TT max tree, cheaper than the
    strided tensor_reduce).
LeakyReLU is one fused DVE scalar_tensor_tensor (max(0.2h, h)); conv5 runs
in float32r (4x PE throughput; post-KNN value path only, so the rounding
cannot flip any neighbor selection).
"""

import numpy as np
from contextlib import ExitStack

import concourse.bass as bass
import concourse.bacc as bacc
import concourse.tile as tile
from concourse import mybir
from concourse.bass_utils import run_bass_kernel_spmd
from concourse.masks import make_identity
import concourse.hw_specs as hw_specs

# The stock cost model budgets 0.34 ns per SWDGE descriptor; hardware measures
# ~8 ns (a 2560-row dma_gather takes ~20.5 us of Pool time). The Tile list
# scheduler simulates with this model to order each engine's stream, and the
# underestimate makes it place gather-dependent DVE work (the k-reduce) far
# too early, stalling the DVE ~20 us per tile. Correct the constant so the
# scheduler sees realistic gather times.
hw_specs.TRN2Spec.SWDGE_NS_PER_DESCRIPTOR = 8.0

F32 = mybir.dt.float32
F32R = mybir.dt.float32r
F16 = mybir.dt.float16
I16 = mybir.dt.int16
U32 = mybir.dt.uint32
AF = mybir.ActivationFunctionType
ALU = mybir.AluOpType
AX = mybir.AxisListType

B, N, KNN, P = 8, 2048, 20, 128
NT = N // P                      # 16 point tiles
EPS = 1e-5
NEG = -1e30
CONV = [(64, 3), (64, 64), (128, 64), (256, 128)]   # (O, C) of edge convs
LIN = [(512, 2048), (256, 512), (128, 256), (2, 128)]
LRELU_CONV = 0.2
LRELU_HEAD = 0.01


def _bn_fold(nc, sb, g_col, b_col, m_col, v_col, ncols, eps_col):
    """s = g * rsqrt(v + eps); t = b - m * s  (all [128, ncols] column tiles)."""
    s = sb.tile([P, ncols], F32, tag="bn_s")
    t = sb.tile([P, ncols], F32, tag="bn_t")
    tmp = sb.tile([P, ncols], F32, tag="bn_tmp")
    nc.scalar.activation(out=tmp, in_=v_col, func=AF.Sqrt, bias=eps_col, scale=1.0)
    nc.vector.reciprocal(out=s, in_=tmp)
    nc.vector.tensor_mul(s, s, g_col)
    nc.vector.tensor_mul(tmp, m_col, s)
    nc.vector.tensor_sub(t, b_col, tmp)
    return s, t


def _emit(nc, tc, t_in, t_w, t_out, dbg):
    with ExitStack() as ctx:
        const = ctx.enter_context(tc.tile_pool(name="const", bufs=1))
        pers = ctx.enter_context(tc.tile_pool(name="pers", bufs=1))

        ident = const.tile([P, P], F32)
        make_identity(nc, ident[:])
        ones_col = const.tile([P, 1], F32)
        nc.vector.memset(ones_col, 1.0)
        ones_row = const.tile([1, P], F32)
        nc.vector.memset(ones_row, 1.0)
        ones_row16 = const.tile([1, P], F16)
        nc.vector.memset(ones_row16, 1.0)
        ones_row16s = const.tile([1, P], F16)
        nc.vector.memset(ones_row16s, 1.0 / 32.0)
        ident16 = const.tile([P, P], F16)
        nc.scalar.activation(out=ident16, in_=ident, func=AF.Copy)
        eps_col = const.tile([P, 1], F32)
        nc.vector.memset(eps_col, EPS)
        # SELR[g][p, p'] = 1 iff p == g*16 + p' % 16  (wrapped-idx builder)
        selr = const.tile([P, 8, P], F32)
        for g in range(8):
            isrc = ident[:, g * 16:(g + 1) * 16]
            src_b = bass.AP(tensor=isrc.tensor, offset=isrc.offset,
                            ap=[isrc.ap[0], [0, 8], isrc.ap[1]])
            nc.vector.tensor_copy(
                out=selr[:, g, :].rearrange("p (o q) -> p o q", q=16), in_=src_b)

        # persistent feature maps (channels-first: [C(part), N(free)])
        x_cf = [
            pers.tile([64, N], F32, tag="x0", name="x0"),
            pers.tile([64, N], F32, tag="x1", name="x1"),
            pers.tile([P, N], F32, tag="x2", name="x2"),
            pers.tile([P, 2 * N], F32, tag="x3", name="x3"),  # 256 ch, 2 chunks
        ]

        def transpose_to(ps_pool, tag, dst_ap, src_ap, rows_out):
            """dst[f, p] = src[p, f] via PE; src SBUF [p<=128, f<=128]."""
            pt = ps_pool.tile([P, P], F32, tag=tag)
            kdim = src_ap.shape[0]
            nc.tensor.transpose(out=pt[0:rows_out, 0:kdim], in_=src_ap,
                                identity=ident[0:kdim, 0:kdim])
            nc.scalar.activation(out=dst_ap, in_=pt[0:rows_out, 0:kdim], func=AF.Copy)

        # ---------------- input transpose: feat [N, 3] -> x_in [3, N] ----------
        with tc.tile_pool(name="ps_setup", bufs=2, space="PSUM") as ps_setup, \
             tc.tile_pool(name="sb_setup", bufs=2) as sb_setup:
            x_in = pers.tile([3, N], F32, tag="x_in")
            for t in range(NT):
                ft = sb_setup.tile([P, 3], F32, tag="feat")
                nc.sync.dma_start(out=ft, in_=t_in["feat_xyz"][t * P:(t + 1) * P, :])
                transpose_to(ps_setup, "tr", x_in[:, t * P:(t + 1) * P], ft[:, :], 3)

        # =================== edge conv layers ===================
        src = x_in
        for li, (O, C) in enumerate(CONV):
            OCH = (O + P - 1) // P  # o-chunks
            with ExitStack() as lctx:
                sb = lctx.enter_context(tc.tile_pool(name=f"sb_l{li}", bufs=1))
                sbw = lctx.enter_context(tc.tile_pool(name=f"sbw_l{li}", bufs=2))
                sbw3 = lctx.enter_context(tc.tile_pool(name=f"sbw3_l{li}", bufs=3))
                kdepth = 3
                GDT = F16 if li == 3 else F32   # fp16 U-table for L3: halves
                # gather buffers (enables depth-3) and doubles k-reduce rate;
                # value-path only (L3 feeds conv5, no further KNN selection).
                sbwg = lctx.enter_context(
                    tc.tile_pool(name=f"sbwg_l{li}", bufs=kdepth + 1))
                ps_s = lctx.enter_context(
                    tc.tile_pool(name=f"ps_s{li}", bufs=3, space="PSUM"))
                ps_sm = lctx.enter_context(
                    tc.tile_pool(name=f"ps_sm{li}", bufs=2, space="PSUM"))
                ps_y = lctx.enter_context(
                    tc.tile_pool(name=f"ps_y{li}", bufs=2, space="PSUM"))

                # --- weight prep: WlT [C, O], WvT = (Wr-Wl)T [C, O]
                wlT = sb.tile([P, O], F32, tag="wlT")
                wvT = sb.tile([P, O], F32, tag="wvT")
                for j in range(OCH):
                    ow = min(P, O - j * P)
                    wsb = sbw.tile([P, 2 * C], F32, tag="w_in")
                    nc.sync.dma_start(out=wsb[0:ow, :],
                                      in_=t_w[f"W{li}"][j * P:j * P + ow, :])
                    transpose_to(ps_sm, "sm", wlT[0:C, j * P:j * P + ow],
                                 wsb[0:ow, 0:C], C)
                    transpose_to(ps_sm, "sm", wvT[0:C, j * P:j * P + ow],
                                 wsb[0:ow, C:2 * C], C)
                nc.vector.tensor_sub(wvT[0:C, 0:O], wvT[0:C, 0:O], wlT[0:C, 0:O])

                # --- BN fold per o-chunk
                g_col = sb.tile([P, OCH], F32, tag="g")
                b_col = sb.tile([P, OCH], F32, tag="b")
                m_col = sb.tile([P, OCH], F32, tag="m")
                v_col = sb.tile([P, OCH], F32, tag="v")
                for j in range(OCH):
                    ow = min(P, O - j * P)
                    for colt, nm in ((g_col, "g"), (b_col, "b"), (m_col, "m"), (v_col, "v")):
                        nc.sync.dma_start(out=colt[0:ow, j:j + 1],
                                          in_=t_w[f"{nm}{li}"][j * P:j * P + ow, :])
                bn_s, bn_t = _bn_fold(nc, sb, g_col, b_col, m_col, v_col, OCH, eps_col)

                # --- per 512-chunk: sq row, fp16 hi/lo splits (chunked so the
                # scheduler can hoist chunk q behind the previous layer's y(q))
                xx = sb.tile([P, N], F32, tag="xx")
                nsq = sb.tile([1, N], F32, tag="nsq")
                xh = sb.tile([C, N], F16, tag="xh")
                xl = sb.tile([C, N], F16, tag="xl")
                nqh = sb.tile([1, N], F16, tag="nqh")
                nql = sb.tile([1, N], F16, tag="nql")
                for q in range(4):
                    sl = slice(q * 512, (q + 1) * 512)
                    nc.scalar.activation(out=xx[0:C, sl], in_=src[0:C, sl],
                                         func=AF.Square)
                    pq = ps_sm.tile([1, 512], F32, tag="sm")
                    nc.tensor.matmul(out=pq, lhsT=ones_col[0:C, :], rhs=xx[0:C, sl],
                                     start=True, stop=True)
                    nc.scalar.activation(out=nsq[:, sl], in_=pq, func=AF.Copy, scale=-0.5)
                    nc.scalar.activation(out=xh[:, sl], in_=src[0:C, sl], func=AF.Copy)
                    nc.vector.tensor_sub(xl[:, sl], src[0:C, sl], xh[:, sl])
                    nc.scalar.activation(out=nqh[:, sl], in_=nsq[:, sl], func=AF.Copy)
                    nc.vector.tensor_sub(nql[:, sl], nsq[:, sl], nqh[:, sl])

                # --- U table -> DRAM
                u_dram = t_w[f"Utab{li}"]
                for t in range(NT):
                    pu = ps_sm.tile([P, 512], F32, tag="sm")
                    nc.tensor.matmul(out=pu[:, 0:O], lhsT=src[0:C, t * P:(t + 1) * P],
                                     rhs=wlT[0:C, 0:O], start=True, stop=True)
                    usb = sbw.tile([P, O], GDT, tag="u_sb")
                    nc.scalar.activation(out=usb, in_=pu[:, 0:O], func=AF.Copy)
                    nc.sync.dma_start(out=u_dram[t * P:(t + 1) * P, :], in_=usb)

                # --- software-pipelined per-tile stages -----------------------
                m_all = sb.tile([P, NT * O], F32, tag="m_all")
                s_tiles = [None] * NT      # s_sb per in-flight tile
                gt_tiles = [None] * NT     # gather output per in-flight tile

                def stage_s(t):
                    """PE: S matrix chunks for tile t; ACT: evacuate to SBUF."""
                    s_sb = sbw3.tile([P, N], F32, tag="s_sb")
                    s_tiles[t] = s_sb
                    tsl = slice(t * P, (t + 1) * P)
                    for q in range(4):
                        sl = slice(q * 512, (q + 1) * 512)
                        pq = ps_s.tile([P, 512], F32, tag="s_ps")
                        nc.tensor.matmul(out=pq, lhsT=xh[:, tsl], rhs=xh[:, sl],
                                         start=True, stop=False)
                        nc.tensor.matmul(out=pq, lhsT=xh[:, tsl], rhs=xl[:, sl],
                                         start=False, stop=False)
                        nc.tensor.matmul(out=pq, lhsT=xl[:, tsl], rhs=xh[:, sl],
                                         start=False, stop=False)
                        nc.tensor.matmul(out=pq, lhsT=ones_row16, rhs=nqh[:, sl],
                                         start=False, stop=False)
                        nc.tensor.matmul(out=pq, lhsT=ones_row16, rhs=nql[:, sl],
                                         start=False, stop=True)
                        nc.scalar.activation(out=s_sb[:, sl], in_=pq, func=AF.Copy)

                def stage_topk(t):
                    """DVE: top-24 + indices; PE: wrapped-idx; Pool: gather."""
                    s_sb = s_tiles[t]
                    v24 = sbw3.tile([P, 24], F32, tag="v24")
                    i24 = sbw3.tile([P, 24], U32, tag="i24")
                    nc.vector.max(out=v24[:, 0:8], in_=s_sb)
                    nc.vector.max_index(out=i24[:, 0:8], in_max=v24[:, 0:8],
                                        in_values=s_sb)
                    nc.vector.match_replace(out=s_sb, in_to_replace=v24[:, 0:8],
                                            in_values=s_sb, imm_value=NEG)
                    nc.vector.max(out=v24[:, 8:16], in_=s_sb)
                    nc.vector.max_index(out=i24[:, 8:16], in_max=v24[:, 8:16],
                                        in_values=s_sb)
                    nc.vector.match_replace(out=s_sb, in_to_replace=v24[:, 8:16],
                                            in_values=s_sb, imm_value=NEG)
                    nc.vector.max(out=v24[:, 16:24], in_=s_sb)
                    nc.vector.max_index(out=i24[:, 16:24], in_max=v24[:, 16:24],
                                        in_values=s_sb)

                    idxf = sbw3.tile([P, KNN], F32, tag="idxf")
                    nc.scalar.activation(out=idxf, in_=i24[:, 0:KNN], func=AF.Copy)
                    if dbg and li == 0:
                        nc.sync.dma_start(
                            out=t_out["dbg_i0"][:, t * KNN:(t + 1) * KNN], in_=idxf)
                    if dbg and li == 0 and t == 0:
                        nc.sync.dma_start(out=t_out["dbg_s0"][:, :], in_=s_sb)
                    pw = ps_sm.tile([P, 8 * KNN], F32, tag="sm")
                    for g in range(8):
                        nc.tensor.matmul(
                            out=pw[:, :].rearrange("p (k g) -> p k g", g=8)[:, :, g],
                            lhsT=selr[:, g, :], rhs=idxf, start=True, stop=True,
                            skip_group_check=True)
                    w16 = sbw3.tile([P, 8 * KNN], I16, tag="w16")
                    nc.scalar.activation(out=w16, in_=pw, func=AF.Copy)

                    gt = sbwg.tile([P, KNN, O], GDT, tag="gather")
                    gt_tiles[t] = gt
                    nc.gpsimd.dma_gather(
                        out_ap=gt[:, :, :], in_ap=u_dram[:, :], idxs_ap=w16[:, :],
                        num_idxs=P * KNN, num_idxs_reg=P * KNN, elem_size=O,
                        single_packet=False)

                def stage_kred(t):
                    """DVE: max over k via in-place TT tree (20->10->5->2+1->1)."""
                    gt = gt_tiles[t]
                    nc.vector.tensor_tensor(out=gt[:, 0:10, :], in0=gt[:, 0:10, :],
                                            in1=gt[:, 10:20, :], op=ALU.max)
                    nc.vector.tensor_tensor(out=gt[:, 0:5, :], in0=gt[:, 0:5, :],
                                            in1=gt[:, 5:10, :], op=ALU.max)
                    nc.vector.tensor_tensor(out=gt[:, 0:2, :], in0=gt[:, 0:2, :],
                                            in1=gt[:, 2:4, :], op=ALU.max)
                    nc.vector.tensor_tensor(out=gt[:, 0:1, :], in0=gt[:, 0:1, :],
                                            in1=gt[:, 1:2, :], op=ALU.max)
                    nc.vector.tensor_tensor(out=m_all[:, t * O:(t + 1) * O],
                                            in0=gt[:, 0, :], in1=gt[:, 4, :],
                                            op=ALU.max)

                # --- y = lrelu(bn(M^T + V)) in channels-first, into next x.
                # Chunk q only needs tiles 4q..4q+3, so it is emitted inside
                # the tile loop right after kred(4q+3).
                dst = x_cf[li]

                def stage_y(q):
                    for j in range(OCH):
                        ow = min(P, O - j * P)
                        py = ps_y.tile([P, 512], F32, tag="y_ps")
                        nc.tensor.matmul(out=py[0:ow, :],
                                         lhsT=wvT[0:C, j * P:j * P + ow],
                                         rhs=src[0:C, q * 512:(q + 1) * 512],
                                         start=True, stop=False)
                        for tt in range(4):
                            t = q * 4 + tt
                            msl = m_all[:, t * O + j * P: t * O + j * P + ow]
                            nc.tensor.matmul(
                                out=py[0:ow, tt * P:(tt + 1) * P],
                                lhsT=msl, rhs=ident,
                                is_transpose=True, start=False, stop=(tt == 3),
                                skip_group_check=True)
                        osl = slice(j * N + q * 512, j * N + (q + 1) * 512)
                        nc.scalar.activation(out=dst[:, osl][0:ow, :],
                                             in_=py[0:ow, :],
                                             func=AF.Identity,
                                             scale=bn_s[0:ow, j:j + 1],
                                             bias=bn_t[0:ow, j:j + 1])
                        nc.vector.scalar_tensor_tensor(
                            out=dst[:, osl][0:ow, :], in0=dst[:, osl][0:ow, :],
                            scalar=LRELU_CONV, in1=dst[:, osl][0:ow, :],
                            op0=ALU.mult, op1=ALU.max)

                stage_s(0)
                stage_s(1)
                for t in range(NT):
                    if t + 2 < NT:
                        stage_s(t + 2)
                    stage_topk(t)
                    if t >= kdepth:
                        stage_kred(t - kdepth)
                        if (t - kdepth) % 4 == 3:
                            stage_y((t - kdepth) // 4)
                for tt in range(NT - kdepth, NT):
                    stage_kred(tt)
                    if tt % 4 == 3:
                        stage_y(tt // 4)
                if dbg:
                    nc.sync.dma_start(out=t_out[f"dbg_x{li}"][:, :], in_=dst[:, :])
            src = x_cf[li]

        # =================== conv5 (1024, f32r) + pooling ===================
        chains = [
            (0, 64, 0, 0),
            (1, 64, 64, 0),
            (2, 128, 128, 0),
            (3, 128, 256, 0),
            (3, 128, 384, N),
        ]
        p_cf = pers.tile([P, 16], F32, tag="p_cf")
        with ExitStack() as cctx:
            sb = cctx.enter_context(tc.tile_pool(name="sb_c5", bufs=1))
            sbw = cctx.enter_context(tc.tile_pool(name="sbw_c5", bufs=2))
            ps_h = cctx.enter_context(tc.tile_pool(name="ps_h", bufs=3, space="PSUM"))
            ps_sm = cctx.enter_context(tc.tile_pool(name="ps_smc", bufs=2, space="PSUM"))

            # f32r-rounded copies of the feature maps
            x_r = [
                sb.tile([64, N], F32R, tag="x0r", name="x0r"),
                sb.tile([64, N], F32R, tag="x1r", name="x1r"),
                sb.tile([P, N], F32R, tag="x2r", name="x2r"),
                sb.tile([P, 2 * N], F32R, tag="x3r", name="x3r"),
            ]
            for xi in range(4):
                nc.scalar.activation(out=x_r[xi][:, :], in_=x_cf[xi][:, :],
                                     func=AF.Copy)

            # W4T per chain: [C_chain, 8*128] tiles (f32r)
            w4T = [sb.tile([P, 1024], F32R, tag=f"w4T_{ci}", name=f"w4T_{ci}")
                   for ci in range(5)]
            for j in range(8):
                wsb = sbw.tile([P, 512], F32, tag="w4_in")
                nc.sync.dma_start(out=wsb, in_=t_w["W4"][j * P:(j + 1) * P, :])
                for ci, (xi, crow, c0, fo) in enumerate(chains):
                    transpose_to(ps_sm, "sm", w4T[ci][0:crow, j * P:(j + 1) * P],
                                 wsb[:, c0:c0 + crow], crow)

            g4 = sb.tile([P, 8], F32, tag="g4")
            b4 = sb.tile([P, 8], F32, tag="b4")
            m4 = sb.tile([P, 8], F32, tag="m4")
            v4 = sb.tile([P, 8], F32, tag="v4")
            for j in range(8):
                for colt, nm in ((g4, "g"), (b4, "b"), (m4, "m"), (v4, "v")):
                    nc.sync.dma_start(out=colt[:, j:j + 1],
                                      in_=t_w[f"{nm}4"][j * P:(j + 1) * P, :])
            s4, t4 = _bn_fold(nc, sb, g4, b4, m4, v4, 8, eps_col)

            scratch = sb.tile([P, 512], F32, tag="scratch")
            for j in range(8):
                h_sb = sbw.tile([P, N], F32, tag="h_sb")
                mean_part = sbw.tile([P, 4], F32, tag="mean_part")
                for q in range(4):
                    ph = ps_h.tile([P, 512], F32, tag="h_ps")
                    for ci, (xi, crow, c0, fo) in enumerate(chains):
                        nc.tensor.matmul(out=ph,
                                         lhsT=w4T[ci][0:crow, j * P:(j + 1) * P],
                                         rhs=x_r[xi][0:crow, fo + q * 512: fo + (q + 1) * 512],
                                         start=(ci == 0), stop=(ci == 4))
                    sl = slice(q * 512, (q + 1) * 512)
                    nc.scalar.activation(out=h_sb[:, sl], in_=ph, func=AF.Identity,
                                         scale=s4[:, j:j + 1], bias=t4[:, j:j + 1])
                    nc.vector.scalar_tensor_tensor(
                        out=h_sb[:, sl], in0=h_sb[:, sl], scalar=LRELU_CONV,
                        in1=h_sb[:, sl], op0=ALU.mult, op1=ALU.max)
                    # mean partial: sum over this chunk
                    nc.scalar.activation(out=scratch, in_=h_sb[:, sl], func=AF.Copy,
                                         accum_out=mean_part[:, q:q + 1])
                # pools
                nc.vector.tensor_reduce(out=p_cf[:, j:j + 1], in_=h_sb[:, :],
                                        axis=AX.X, op=ALU.max)
                nc.vector.tensor_reduce(out=p_cf[:, 8 + j:9 + j], in_=mean_part[:, :],
                                        axis=AX.X, op=ALU.add)
            nc.vector.tensor_scalar_mul(p_cf[:, 8:16], p_cf[:, 8:16], 1.0 / N)
            if dbg:
                nc.sync.dma_start(out=t_out["dbg_p"][:, :], in_=p_cf[:, :])

        # =================== MLP head (broadcast + DVE dot-products) ==========
        with ExitStack() as hctx:
            sb = hctx.enter_context(tc.tile_pool(name="sb_head", bufs=1))
            sbw = hctx.enter_context(tc.tile_pool(name="sbw_head", bufs=2))
            ps_hd = hctx.enter_context(tc.tile_pool(name="ps_hd", bufs=2, space="PSUM"))

            def lin(name, src_col, incols, w_dram, out_dim, alpha):
                """dst [128, ceil(out/128)] = lrelu(alpha)(W @ src).
                src_col [128, incols] column tile (in_dim = 128*incols)."""
                in_dim = P * incols
                och = (out_dim + P - 1) // P
                orows = min(P, out_dim)
                # broadcast src over partitions: bcast[p', c] = src[c]
                bcast = sb.tile([P, in_dim], F32, tag=f"{name}_bc")
                for j in range(incols):
                    pT = ps_hd.tile([1, P], F32, tag="hd_tr")
                    nc.tensor.transpose(out=pT, in_=src_col[:, j:j + 1],
                                        identity=ident)
                    rowj = sbw.tile([1, P], F32, tag="hd_row")
                    nc.scalar.activation(out=rowj, in_=pT, func=AF.Copy)
                    pb = ps_hd.tile([P, P], F32, tag="hd_bc")
                    nc.tensor.matmul(out=pb, lhsT=ones_row, rhs=rowj,
                                     start=True, stop=True)
                    nc.scalar.activation(out=bcast[:, j * P:(j + 1) * P], in_=pb,
                                         func=AF.Copy)
                dst = sb.tile([P, och], F32, tag=f"{name}_out")
                for ot in range(och):
                    orw = min(P, out_dim - ot * P)
                    wsb = sbw.tile([P, in_dim], F32, tag=f"{name}_w")
                    nc.sync.dma_start(out=wsb[0:orw, :],
                                      in_=w_dram[ot * P:ot * P + orw, :])
                    prod = sbw.tile([P, in_dim], F32, tag=f"{name}_prod")
                    nc.vector.tensor_mul(prod[0:orw, :], wsb[0:orw, :], bcast[0:orw, :])
                    nc.vector.tensor_reduce(out=dst[0:orw, ot:ot + 1],
                                            in_=prod[0:orw, :], axis=AX.X, op=ALU.add)
                if alpha is not None:
                    nc.vector.scalar_tensor_tensor(
                        out=dst[0:orows, :], in0=dst[0:orows, :], scalar=alpha,
                        in1=dst[0:orows, :], op0=ALU.mult, op1=ALU.max)
                return dst

            y1 = lin("y1", p_cf, 16, t_w["L1"], 512, LRELU_HEAD)
            y2 = lin("y2", y1, 4, t_w["L2"], 256, LRELU_HEAD)
            y3 = lin("y3", y2, 2, t_w["L3"], 128, LRELU_HEAD)
            y4 = lin("y4", y3, 1, t_w["L4"], 2, None)
            osb = sb.tile([2, 1], F32, tag="out_sb")
            nc.vector.tensor_copy(out=osb, in_=y4[0:2, 0:1])
            nc.sync.dma_start(out=t_out["out"][:, :], in_=osb)


_PROG_CACHE = {}


def _build(dbg=False):
    key = ("v5_final", dbg)
    if key in _PROG_CACHE:
        return _PROG_CACHE[key]
    nc = bacc.Bacc("TRN2", target_bir_lowering=False, debug=False, num_devices=B)
    t_in = {"feat_xyz": nc.declare_dram_parameter("feat_xyz", [N, 3], F32, isOutput=False)}
    t_w = {}
    for li, (O, C) in enumerate(CONV + [(1024, 512)]):
        wshape = [O, 2 * C] if li < 4 else [O, C]
        t_w[f"W{li}"] = nc.declare_dram_parameter(f"W{li}", wshape, F32, isOutput=False)
        for nm in "gbmv":
            t_w[f"{nm}{li}"] = nc.declare_dram_parameter(f"{nm}{li}", [O, 1], F32,
                                                         isOutput=False)
    for j, (o, c) in enumerate(LIN):
        t_w[f"L{j+1}"] = nc.declare_dram_parameter(f"L{j+1}", [o, c], F32, isOutput=False)
    for li, (O, C) in enumerate(CONV):
        t_w[f"Utab{li}"] = nc.dram_tensor(f"Utab{li}", [N, O],
                                          F16 if li == 3 else F32)
    t_out = {"out": nc.declare_dram_parameter("out", [2, 1], F32, isOutput=True)}
    if dbg:
        for li, (O, C) in enumerate(CONV):
            sh = [P, 2 * N] if O == 256 else [O, N]
            t_out[f"dbg_x{li}"] = nc.declare_dram_parameter(f"dbg_x{li}", sh, F32,
                                                            isOutput=True)
        t_out["dbg_p"] = nc.declare_dram_parameter("dbg_p", [P, 16], F32, isOutput=True)
        t_out["dbg_i0"] = nc.declare_dram_parameter("dbg_i0", [P, NT * KNN], F32,
                                                    isOutput=True)
        t_out["dbg_s0"] = nc.declare_dram_parameter("dbg_s0", [P, N], F32,
                                                    isOutput=True)

    with tile.TileContext(nc) as tc:
        _emit(nc, tc, t_in, t_w, t_out, dbg)
    nc.compile()
    _PROG_CACHE[key] = nc
    return nc


def _make_in_maps(inputs):
    feat = np.ascontiguousarray(np.asarray(inputs["feat_xyz"], dtype=np.float32))
    common = {}
    for li in range(5):
        common[f"W{li}"] = np.ascontiguousarray(np.asarray(inputs[f"W{li}"], np.float32))
        for nm in "gbmv":
            common[f"{nm}{li}"] = np.ascontiguousarray(
                np.asarray(inputs[f"{nm}{li}"], np.float32).reshape(-1, 1))
    for j in range(1, 5):
        common[f"L{j}"] = np.ascontiguousarray(np.asarray(inputs[f"L{j}"], np.float32))
    return [dict(common, feat_xyz=np.ascontiguousarray(feat[b])) for b in range(B)]


def run(inputs, dbg=False, trace=False, **kw):
    nc = _build(dbg)
    in_maps = _make_in_maps(inputs)
    return run_bass_kernel_spmd(nc, in_maps, list(range(B)), trace=trace, **kw)


def kernel(**inputs):
    res = run(inputs).results
    out = np.stack([res[b]["out"][:, 0] for b in range(B)], axis=0)
    return out.astype(np.float32)
